# revision 1
# baseline (speedup 1.0000x reference)
"""KShape distance kernel for Trainium2 (8 NeuronCores, Bass/Tile).

Math: dists[b,k] = max_lag( sum_{t,d} x[b,t,d]*c[k,t-l,d] ) / (||x_b|| * k)
for k>=1 (k=0 -> 0), lags l in [-511,511]; labels = argmax_k dists.

Device strategy (K-sharded, 2 clusters per core, SPMD identical program):
  - contraction rows r=(t,d) (4096) on partitions, 32 chunks of 128
  - lhsT = x^T chunk [128 rows, 128 b]  (x pre-transposed on host: layout only)
  - rhs  = sliding window W[u, j] = cpad[s, dt+j, d] (u=16dt+? see below);
    chunk ci & lag l'' read W[:, 16ci+l''], so ONE [128,1520] window tile per
    cluster serves all 32 chunks x 1024 lags via AP slicing (no Toeplitz
    materialization). l''=0 corresponds to the zero-overlap circular lag the
    reference excludes -> excluded from the max.
  - fp32r matmuls (1 cyc/row, ~1.6e-4 rel err), PSUM accum over 32 chunks,
    8 concurrent streams = (2 clusters x 2 b-tiles x 2 lag-halves).
  - max over lags on DVE; sum(x^2) per sample via ACT Square + DVE reduce.
Host: shard/marshal inputs, then dists = mx/(norms*k), labels = argmax.
"""
import numpy as np

B, T, D, K = 256, 512, 8, 16
R = T * D            # 4096 contraction rows
NCORES = 8
PAD = 1536           # padded center length (lag window needs 1535)
WJ = 1520            # window free size: j in [0,1520), m = dt + j <= 15+1519 = 1534

_CACHE = {}


def _build_nc():
    import concourse.tile as tile
    from concourse import bacc, mybir

    nc = bacc.Bacc("TRN2", target_bir_lowering=False, debug=False, num_devices=NCORES)
    xt_d = nc.declare_dram_parameter("xt", [R, B], mybir.dt.float32, isOutput=False)
    x_d = nc.declare_dram_parameter("x", [B, R], mybir.dt.float32, isOutput=False)
    cpad_d = nc.declare_dram_parameter("cpad", [2, PAD, D], mybir.dt.float32, isOutput=False)
    mx_d = nc.declare_dram_parameter("mx", [2, B], mybir.dt.float32, isOutput=True)
    ssq_d = nc.declare_dram_parameter("ssq", [B], mybir.dt.float32, isOutput=True)

    import concourse.bass as bass

    with tile.TileContext(nc) as tc:
        with (
            tc.tile_pool(name="sb", bufs=1) as sb,
            tc.tile_pool(name="ps", bufs=8, space="PSUM") as pspool,
        ):
            # ---- load x^T (32 chunks side by side) and round to fp32r ----
            xt_f = sb.tile([128, 32 * 256], mybir.dt.float32, tag="xt_f")
            for ci in range(32):
                nc.sync.dma_start(
                    xt_f[:, ci * 256:(ci + 1) * 256],
                    xt_d.ap()[ci * 128:(ci + 1) * 128, :],
                )
            xt_r = sb.tile([128, 32 * 256], mybir.dt.float32r, tag="xt_r")
            nc.vector.tensor_copy(xt_r[:], xt_f[:])

            # ---- lag-window tiles per cluster slot: W[u=(dt,d), j] = cpad[s, dt+j, d]
            w_r = []
            for s in range(2):
                wf = sb.tile([128, WJ], mybir.dt.float32, tag=f"w_f{s}")
                src = bass.AP(cpad_d, s * PAD * D, [[D, 16], [1, D], [D, WJ]])
                nc.sync.dma_start(wf[:], src)
                wr = sb.tile([128, WJ], mybir.dt.float32r, tag=f"w_r{s}")
                nc.vector.tensor_copy(wr[:], wf[:])
                w_r.append(wr)

            # ---- main correlation streams ----
            for s in range(2):
                for bt in range(2):
                    mxpair = sb.tile([128, 2], mybir.dt.float32, tag=f"mxp{s}{bt}")
                    for h in range(2):
                        ps = pspool.tile([128, 512], mybir.dt.float32, tag="ps")
                        for ci in range(32):
                            nc.tensor.matmul(
                                ps[:],
                                xt_r[:, ci * 256 + bt * 128: ci * 256 + bt * 128 + 128],
                                w_r[s][:, 16 * ci + 512 * h: 16 * ci + 512 * h + 512],
                                start=(ci == 0),
                                stop=(ci == 31),
                            )
                        lo = 1 if h == 0 else 0  # exclude l''=0 (zero-overlap lag)
                        nc.vector.tensor_reduce(
                            mxpair[:, h:h + 1], ps[:, lo:512],
                            axis=mybir.AxisListType.X, op=mybir.AluOpType.max,
                        )
                    mxf = sb.tile([128, 1], mybir.dt.float32, tag=f"mxf{s}{bt}")
                    nc.vector.tensor_reduce(
                        mxf[:], mxpair[:], axis=mybir.AxisListType.X,
                        op=mybir.AluOpType.max,
                    )
                    nc.sync.dma_start(mx_d.ap()[s, bt * 128:(bt + 1) * 128], mxf[:])

            # ---- sum of squares per sample (for host-side ||x_b||) ----
            for bt in range(2):
                xn = sb.tile([128, R], mybir.dt.float32, tag=f"xn{bt}")
                nc.sync.dma_start(xn[:], x_d.ap()[bt * 128:(bt + 1) * 128, :])
                sq = sb.tile([128, R], mybir.dt.float32, tag=f"sq{bt}")
                nc.scalar.activation(sq[:], xn[:], mybir.ActivationFunctionType.Square)
                ss = sb.tile([128, 1], mybir.dt.float32, tag=f"ss{bt}")
                nc.vector.tensor_reduce(
                    ss[:], sq[:], axis=mybir.AxisListType.X, op=mybir.AluOpType.add,
                )
                nc.sync.dma_start(ssq_d.ap()[bt * 128:(bt + 1) * 128], ss[:])
    nc.compile()
    return nc


def kernel(x, cluster_centers):
    from concourse.bass_utils import run_bass_kernel_spmd

    if "nc" not in _CACHE:
        _CACHE["nc"] = _build_nc()
    nc = _CACHE["nc"]

    x = np.asarray(x, dtype=np.float32)
    c = np.asarray(cluster_centers, dtype=np.float32)
    xf = np.ascontiguousarray(x.reshape(B, R))          # [256, 4096]
    xtf = np.ascontiguousarray(xf.T)                    # [4096, 256]

    in_maps = []
    for core in range(NCORES):
        ks = [2 * core + 1, min(2 * core + 2, K - 1)]   # core 7 -> [15, 15]
        cpad = np.zeros((2, PAD, D), dtype=np.float32)
        for s, k in enumerate(ks):
            cpad[s, T:2 * T, :] = c[k]
        in_maps.append({"xt": xtf, "x": xf, "cpad": cpad})

    res = run_bass_kernel_spmd(nc, in_maps, list(range(NCORES)))

    ssq = res.results[0]["ssq"].astype(np.float32)
    norms = np.sqrt(ssq)
    dists = np.zeros((B, K), dtype=np.float32)
    for k in range(1, K):
        core, slot = (k - 1) // 2, (k - 1) % 2
        dists[:, k] = res.results[core]["mx"][slot] / (norms * np.float32(k))
    labels = np.argmax(dists, axis=1).astype(np.int32)
    return (labels, dists)
